# revision 20
# baseline (speedup 1.0000x reference)
"""Trainium2 Bass kernel for nn_DendriticLinear.

The reference simulates RESOLUTION=10 steps of a linear dynamical system on
state tensors of shape (B, OUT, IN) and returns only soma (B, OUT).  Because
the dynamics are linear in the states and in inject = x*W*dt (the sigmoids
only touch step-constant parameters), soma factors exactly:

    soma[b, o] = sum_i x[b, i] * Meff[o, i]
    Meff[o, i] = dt * W[o, i] * m[o, i]

where m comes from a batch-independent adjoint recurrence over the (OUT, IN)
parameter grid only (verified exact in fp64 against the forward simulation).
With sc = 2*sigmoid(space), tau = 2*sigmoid(time), D = 2*dt*sigmoid(decay)
(per OUT row), A = tau - coef*sc, P = D*A, Q = D*sc and the truncated
neighbour-shift S(v)_i = v_{i-1} + v_{i+1}:

    g_0 = sc ; lic_0 = sc ; m = 10*sc
    for i = 1..9:
        g_i   = t2_{i-1} + h_{i-1}         with  h = P*g + sc  (computed from
        lic  += g_i                              the previous g off the
        m    += (10-i) * g_i                     critical path)
        t2_i  = Q * S(lic)
    (final iteration needs only g and the m update)

Sharding: OUT rows are split across the 8 cores (64 rows each).  Inside a
core the 64x512 grid is folded onto 128 SBUF partitions as two overlapping
IN-halves (cols [0:272) and [240:512)); the 10-step neighbour coupling only
travels 10 columns, so each half computes its 256 owned columns exactly with
no cross-partition (or cross-core) traffic.  Ghost zero columns at both ends
of the lam_ic buffer make the truncated shift two plain shifted ops.

Engine notes (measured): GpSimd shares the SBUF port with the Vector engine,
so offloading loop ops to GpSimd halves the throughput of concurrent DVE ops
— the whole recurrence therefore runs on DVE alone, in bf16 (2x packed DVE
mode, ~292 ns per 128x272 op) with only the m accumulator kept in fp32.
A dummy 1-element sigmoid issued at kernel start pulls the ~1.3 us
ACT-table load off the phase-0 critical path.
"""

import numpy as np

B, OUT, IN = 64, 512, 512
K = 10
DT = 0.001
NCORES = 8
RPC = OUT // NCORES          # out rows per core = 64
HW = 272                     # folded half width (256 owned + overlap)
OFF_B = IN - HW              # 240: start column of the second half
VB = 256 - OFF_B             # 16: first owned column of the second half

_cached = None


def _build_bass():
    import concourse.mybir as mybir
    from concourse import bacc, masks
    from concourse.tile import TileContext

    f32 = mybir.dt.float32
    Alu = mybir.AluOpType
    Act = mybir.ActivationFunctionType

    nc = bacc.Bacc()
    x_h = nc.dram_tensor("x", [B, IN], f32, kind="ExternalInput")
    w_h = nc.dram_tensor("w", [RPC, IN], f32, kind="ExternalInput")
    tc_h = nc.dram_tensor("tcon", [RPC, IN], f32, kind="ExternalInput")
    sp_h = nc.dram_tensor("scon", [RPC, IN], f32, kind="ExternalInput")
    dd_h = nc.dram_tensor("dd", [RPC, 1], f32, kind="ExternalInput")
    out_h = nc.dram_tensor("soma", [B, RPC], f32, kind="ExternalOutput")

    with TileContext(nc) as tc:
        with (
            tc.tile_pool(name="main", bufs=1) as pool,
            tc.tile_pool(name="psum", bufs=2, space="PSUM") as ppool,
        ):
            # ---- loads, parameter tensors first (folded layout: partitions
            #      0:64 = cols [0:HW), partitions 64:128 = cols [OFF_B:IN)) ----
            def folded_load(dst_tile, src_h):
                nc.sync.dma_start(dst_tile[0:RPC, :], src_h[:, 0:HW])
                nc.sync.dma_start(dst_tile[RPC:128, :], src_h[:, OFF_B:IN])

            # dummy sigmoid: forces the ACT function-table load to happen
            # immediately instead of gating the first real sigmoid later
            warm = pool.tile([1, 1], f32)
            nc.vector.memset(warm[:], 0.0)
            nc.scalar.activation(warm[:], warm[:], Act.Sigmoid)

            spf = pool.tile([128, HW], f32)
            folded_load(spf, sp_h)
            ddf = pool.tile([128, 1], f32)
            nc.sync.dma_start(ddf[0:RPC, :], dd_h[:, :])
            nc.sync.dma_start(ddf[RPC:128, :], dd_h[:, :])
            tcf = pool.tile([128, HW], f32)
            folded_load(tcf, tc_h)
            wf = pool.tile([128, HW], f32)
            folded_load(wf, w_h)
            xa = pool.tile([B, IN], f32)
            nc.sync.dma_start(xa[:], x_h[:])

            # ---- parameters ----
            s1 = pool.tile([128, HW], f32)   # sigmoid(space)
            s2 = pool.tile([128, HW], f32)   # sigmoid(time)
            sc = pool.tile([128, HW], f32)   # 2*sigmoid(space)
            tau = pool.tile([128, HW], f32)  # 2*sigmoid(time)
            nA = pool.tile([128, HW], f32)   # 2*sc - tau = -A
            P = pool.tile([128, HW], f32)    # D*A
            Q = pool.tile([128, HW], f32)    # D*sc
            dvec = pool.tile([128, 1], f32)  # 2*dt*sigmoid(dd)
            s3 = pool.tile([128, 1], f32)
            nc.scalar.activation(s1[:], spf[:], Act.Sigmoid)
            nc.scalar.activation(s3[:], ddf[:], Act.Sigmoid)
            nc.scalar.activation(s2[:], tcf[:], Act.Sigmoid)
            nc.vector.tensor_scalar_mul(dvec[:], s3[:], 2.0 * DT)
            nc.vector.tensor_scalar_mul(sc[:], s1[:], 2.0)
            nc.vector.tensor_scalar_mul(tau[:], s2[:], 2.0)
            nc.vector.scalar_tensor_tensor(nA[:], sc[:], 2.0, tau[:],
                                           Alu.mult, Alu.subtract)
            # boundary coefficient fixups at the two true edges
            nc.vector.tensor_sub(nA[0:RPC, 0:1], nA[0:RPC, 0:1], sc[0:RPC, 0:1])
            nc.vector.tensor_sub(nA[RPC:128, HW - 1:HW], nA[RPC:128, HW - 1:HW],
                                 sc[RPC:128, HW - 1:HW])
            # P = D*A = (nA * D) * -1 ; Q = D*sc
            nc.vector.tensor_scalar(P[:], nA[:], dvec[:], -1.0,
                                    Alu.mult, Alu.mult)
            nc.vector.tensor_scalar(Q[:], sc[:], dvec[:], None, Alu.mult)
            # bf16 copies of the loop coefficients: the recurrence runs in
            # bf16 (2x DVE mode), only the m accumulator stays fp32
            bf16 = mybir.dt.bfloat16
            scb = pool.tile([128, HW], bf16)
            Pb = pool.tile([128, HW], bf16)
            Qb = pool.tile([128, HW], bf16)
            nc.vector.tensor_copy(scb[:], sc[:])
            nc.vector.tensor_copy(Pb[:], P[:])
            nc.vector.tensor_copy(Qb[:], Q[:])

            # ---- transpose x early (PE runs while DVE does the recurrence) ----
            ident = pool.tile([128, 128], f32)
            masks.make_identity(nc, ident[:])
            xT = pool.tile([128, 4 * B], f32)
            for c in range(4):
                pt = ppool.tile([128, B], f32, tag="tpsum")
                nc.tensor.transpose(pt[:], xa[:, c * 128:(c + 1) * 128],
                                    ident[0:B, 0:B])
                nc.vector.tensor_copy(xT[:, c * B:(c + 1) * B], pt[:])

            # ---- adjoint recurrence (bf16 state, fp32 m accumulator) ----
            licb = pool.tile([128, HW + 2], bf16)  # ghost col 0 and HW+1 stay 0
            m = pool.tile([128, HW], f32)
            g = pool.tile([128, HW], bf16)
            u = pool.tile([128, HW], bf16)
            t2 = pool.tile([128, HW], bf16)
            z = pool.tile([128, HW], bf16)
            h = pool.tile([128, HW], bf16)
            nc.vector.memset(licb[:, 0:1], 0.0)
            nc.vector.memset(licb[:, HW + 1:HW + 2], 0.0)

            lic = licb[:, 1:HW + 1]
            licL = licb[:, 0:HW]
            licR = licb[:, 2:HW + 2]

            def shift_and_t2():
                # u = S(lic) as a single dual-read add (full rate when no
                # GpSimd op contends for the shared SBUF port), then
                # t2 = Q*u via the full-rate STT multiply path
                nc.vector.tensor_add(u[:], licL, licR)
                nc.vector.tensor_mul(t2[:], Qb[:], u[:])

            # i = 0: all states are zero, so g_0 = sc
            nc.vector.tensor_copy(lic, scb[:])
            nc.vector.tensor_scalar_mul(m[:], sc[:], float(K))
            nc.vector.tensor_mul(z[:], Pb[:], scb[:])
            nc.vector.tensor_add(h[:], z[:], scb[:])
            shift_and_t2()

            for i in range(1, K - 1):
                nc.vector.tensor_add(g[:], t2[:], h[:])
                nc.vector.tensor_add(lic, lic, g[:])
                nc.vector.tensor_mul(z[:], Pb[:], g[:])
                nc.vector.scalar_tensor_tensor(m[:], g[:], float(K - i), m[:],
                                               Alu.mult, Alu.add)
                nc.vector.tensor_add(h[:], z[:], scb[:])
                shift_and_t2()

            # i = K-1: only g and the m update are needed
            nc.vector.tensor_add(g[:], t2[:], h[:])
            nc.vector.tensor_add(m[:], m[:], g[:])

            # ---- Meff = (m * dt) * W, transposed chunks put IN on partitions ----
            meff = pool.tile([128, HW], f32)
            nc.vector.scalar_tensor_tensor(meff[:], m[:], DT, wf[:],
                                           Alu.mult, Alu.mult)
            mT = pool.tile([128, 4 * RPC], f32)
            chunks = ((0, 0), (0, 128), (RPC, VB), (RPC, VB + 128))
            for c, (pr, co) in enumerate(chunks):
                pt2 = ppool.tile([128, RPC], f32, tag="tpsum")
                # identity block must share the lhsT base partition; the
                # bottom-right quadrant of I_128 is I_64 at base partition 64
                idb = ident[pr:pr + RPC, pr:pr + RPC]
                nc.tensor.transpose(pt2[:], meff[pr:pr + RPC, co:co + 128],
                                    idb)
                nc.vector.tensor_copy(mT[:, c * RPC:(c + 1) * RPC], pt2[:])

            # ---- soma[b, o] = sum_i xT[i, b] * mT[i, o] ----
            acc = ppool.tile([B, RPC], f32, tag="acc")
            for c in range(4):
                nc.tensor.matmul(acc[:], xT[:, c * B:(c + 1) * B],
                                 mT[:, c * RPC:(c + 1) * RPC],
                                 start=(c == 0), stop=(c == 3))
            outt = pool.tile([B, RPC], f32)
            nc.vector.tensor_copy(outt[:], acc[:])
            nc.sync.dma_start(out_h[:], outt[:])

    nc.finalize()
    return nc


def _get_nc():
    global _cached
    if _cached is None:
        _cached = _build_bass()
    return _cached


def kernel(x, dendrite_weights, time_constants, space_constants, dend_decay):
    from concourse.bass_utils import run_bass_kernel_spmd

    x = np.ascontiguousarray(np.asarray(x, dtype=np.float32))
    W = np.ascontiguousarray(np.asarray(dendrite_weights, dtype=np.float32))
    tcn = np.ascontiguousarray(np.asarray(time_constants, dtype=np.float32))
    spc = np.ascontiguousarray(np.asarray(space_constants, dtype=np.float32))
    dd = np.ascontiguousarray(np.asarray(dend_decay, dtype=np.float32))

    nc = _get_nc()
    in_maps = []
    for c in range(NCORES):
        r = slice(c * RPC, (c + 1) * RPC)
        in_maps.append({
            "x": x,
            "w": np.ascontiguousarray(W[r]),
            "tcon": np.ascontiguousarray(tcn[r]),
            "scon": np.ascontiguousarray(spc[r]),
            "dd": np.ascontiguousarray(dd[r]),
        })
    res = run_bass_kernel_spmd(nc, in_maps, core_ids=list(range(NCORES)))
    soma = np.empty((B, OUT), dtype=np.float32)
    for c in range(NCORES):
        soma[:, c * RPC:(c + 1) * RPC] = res.results[c]["soma"]
    return soma
